# revision 1
# baseline (speedup 1.0000x reference)
"""Trainium2 Bass/Tile kernel for causal self-attention (16 heads, T=2048, D=128).

Sharding: tensor-parallel over heads across 8 cores (2 heads/core).
 - QKV projection, RMSNorm, RoPE, lambda-mix, attention: per-head local.
 - y (pre-output-projection activations, [d, t] layout) AllGather'd across
   cores on the head axis, chunk-wise (4 x 512-column AllGathers) so the
   output projection pipelines behind attention.
 - Output projection sharded over output-channel columns: core i computes
   out[:, 256*i : 256*(i+1)] (transposed on device, un-transposed on host).

Matmuls run in float32r (fp32 storage, PE rounds to ~11-bit mantissa,
1 cyc/row at N>=256) — measured rel err ~1.5e-4 per dot on HW.

Softmax uses no max-subtraction: RMS-normed q,k bound |scores| <= 0.12*128,
so exp never overflows fp32; row sums are produced by a ones-vector matmul
and divided out at the end (flash-attention style).

Self-contained: hardcodes all shapes; no reads of /root/problem/*.
"""

from contextlib import ExitStack

import ml_dtypes
import numpy as np

import concourse.bass as bass
import concourse.mybir as mybir
import concourse.tile as tile
from concourse import bacc
from concourse.bass_utils import run_bass_kernel_spmd
from concourse.masks import make_identity

F32 = mybir.dt.float32
F32R = mybir.dt.float32r
BF16 = mybir.dt.bfloat16
MDT = F32R  # matmul operand dtype (q/k/v/score path)
ALU = mybir.AluOpType
ACTF = mybir.ActivationFunctionType

T = 2048          # sequence length
DIM = 2048        # model dim
D = 128           # head dim
NCORES = 8
HPC = 2           # heads per core
HD = HPC * D      # local head-dim cols (256)
TT = T // 128     # t tiles (16)
CT = DIM // 128   # c tiles (16)
NCH = T // 512    # 512-wide t chunks (4)
ATTN_SCALE = 0.12
EPS = 1e-6
RSQRT_D = float(1.0 / np.sqrt(128.0))


def _build_program(sim_mode=False):
    nc = bacc.Bacc(
        "TRN2",
        target_bir_lowering=False,
        debug=False,
        num_devices=NCORES,
    )

    xT_d = nc.dram_tensor("xT", [DIM, T], MDT, kind="ExternalInput")
    wqkvT_d = nc.dram_tensor("wqkvT", [DIM, 3 * HD], MDT, kind="ExternalInput")
    ve_s_d = nc.dram_tensor("ve_s", [T, HD], F32, kind="ExternalInput")
    woT_d = nc.dram_tensor("woT", [DIM, HD], BF16, kind="ExternalInput")
    lam_d = nc.dram_tensor("lam", [1, 2], F32, kind="ExternalInput")
    cos_d = nc.dram_tensor("cos2_pk", [128, TT * 128], F32, kind="ExternalInput")
    sin_d = nc.dram_tensor("sin2_pk", [128, TT * 128], F32, kind="ExternalInput")
    mask_d = nc.dram_tensor("masktri", [128, 128], MDT, kind="ExternalInput")
    outT_d = nc.dram_tensor("outT", [HD, T], F32, kind="ExternalOutput")

    rg = [list(range(NCORES))]

    with tile.TileContext(nc) as tc:
        with (
            tc.tile_pool(name="const", bufs=1) as constp,
            tc.tile_pool(name="persist", bufs=1) as persist,
            tc.tile_pool(name="wqkv", bufs=1) as wqp,
            tc.tile_pool(name="wo", bufs=1) as wop,
            tc.tile_pool(name="dram", bufs=1, space="DRAM") as dramp,
        ):
            # ---- constants (cos/sin DMAs deferred past first weights) ----
            cos_sb = constp.tile([128, TT * 128], F32)
            sin_sb = constp.tile([128, TT * 128], F32)
            mask_sb = constp.tile([128, 128], MDT)
            ones_f32 = constp.tile([128, 128], F32)
            nc.vector.memset(ones_f32, 1.0)
            ones_col = constp.tile([128, 1], MDT)
            nc.vector.tensor_copy(ones_col, ones_f32[:, 0:1])
            ones_row = constp.tile([1, 128], MDT)
            nc.vector.tensor_copy(ones_row, ones_f32[0:1, :])
            ident = constp.tile([128, 128], F32)
            make_identity(nc, ident)
            lam_bc = constp.tile([128, 1], F32)
            nc.sync.dma_start(out=lam_bc, in_=lam_d[0:1, 0:1].to_broadcast([128, 1]))
            eps_sb = constp.tile([128, 1], F32)
            nc.vector.memset(eps_sb, EPS)
            zero_sb = constp.tile([128, 1], F32)
            nc.vector.memset(zero_sb, 0.0)

            # ---- weight tiles (coalesced: 4 cts per DMA) ----
            wq_t = [wqp.tile([128, 4, 3 * HD], MDT, name=f"wq{i}")
                    for i in range(4)]
            wq_src = wqkvT_d.rearrange("(a b p) e -> a p b e", a=4, b=4, p=128)

            # ---- persistent activations (per-region tiles) ----
            qT_t = [[persist.tile([128, 512], MDT, name=f"qT{h}_{c}")
                     for c in range(NCH)] for h in range(HPC)]
            kT_t = [[persist.tile([128, 512], MDT, name=f"kT{h}_{c}")
                     for c in range(NCH)] for h in range(HPC)]
            v_t = [persist.tile([128, HD], MDT, name=f"v{g}")
                   for g in range(TT)]
            yT_t = [[persist.tile([128, 512], BF16, name=f"yT{h}_{c}")
                     for c in range(NCH)] for h in range(HPC)]

            # per-chunk DRAM tiles for the pipelined AllGather
            y_dram = [dramp.tile([HD, 512], BF16, name=f"ydr{c}")
                      for c in range(NCH)]
            yfull = [dramp.tile([DIM, 512], BF16, addr_space="Shared",
                                name=f"yfl{c}")
                     for c in range(NCH)]

            # =========== P1: qkv projection + norm + rope + mix =============
            with ExitStack() as pools:
                ep = pools.enter_context
                xtp = ep(tc.tile_pool(name="p1x", bufs=1))
                vep = ep(tc.tile_pool(name="p1ve", bufs=2))
                psq_p = ep(tc.tile_pool(name="p1ps", bufs=1, space="PSUM"))
                evp = ep(tc.tile_pool(name="p1ev", bufs=3))
                statp = ep(tc.tile_pool(name="p1stat", bufs=4))
                rp = ep(tc.tile_pool(name="p1r", bufs=2))
                st_p = ep(tc.tile_pool(name="p2st", bufs=2, space="PSUM"))
                y_p = ep(tc.tile_pool(name="p2y", bufs=1, space="PSUM"))
                l_p = ep(tc.tile_pool(name="p2l", bufs=1, space="PSUM"))
                o_p = ep(tc.tile_pool(name="p4ps", bufs=1, space="PSUM"))
                e_p = ep(tc.tile_pool(name="p2e", bufs=4))
                rl_p = ep(tc.tile_pool(name="p2rl", bufs=2))
                yf_p = ep(tc.tile_pool(name="p4yf", bufs=3))
                ob_p = ep(tc.tile_pool(name="p4ob", bufs=2))
                # output-projection weights (first needed by P4 chunk 0)
                wo_t = [wop.tile([128, 4, HD], BF16, name=f"wo{i}")
                        for i in range(4)]
                wo_src = woT_d.rearrange("(a b p) e -> a p b e", a=4, b=4,
                                         p=128)

                def emit_p1(sg):
                    xts = []
                    xt_src = xT_d.rearrange(
                        "(ci cj p) t -> ci p cj t", ci=4, cj=4, p=128)
                    for ci in range(4):
                        xt = xtp.tile([128, 4, 256], MDT, name=f"xt{ci}")
                        if sg == 0 and ci == 0:
                            # fine-grained first group so the PE starts ASAP
                            for cj in range(4):
                                nc.sync.dma_start(
                                    out=wq_t[0][:, cj, :],
                                    in_=wq_src[0][:, cj, :])
                                nc.sync.dma_start(
                                    out=xt[:, cj, :],
                                    in_=xt_src[0, :, cj, 0:256])
                        else:
                            if sg == 0:
                                nc.sync.dma_start(out=wq_t[ci],
                                                  in_=wq_src[ci])
                            nc.sync.dma_start(
                                out=xt,
                                in_=xt_src[ci, :, :, sg * 256:(sg + 1) * 256],
                            )
                        xts.append(xt)
                    if sg == 0:
                        nc.sync.dma_start(out=cos_sb, in_=cos_d[:, :])
                        nc.sync.dma_start(out=sin_sb, in_=sin_d[:, :])
                        nc.sync.dma_start(out=mask_sb, in_=mask_d[:, :])
                    ve_t = vep.tile([128, 2, HD], F32, name="ve_t")
                    nc.sync.dma_start(
                        out=ve_t,
                        in_=ve_s_d.rearrange("(a two p) d -> a p two d",
                                             two=2, p=128)[sg],
                    )
                    for gi in range(2):
                        g = sg * 2 + gi
                        ps = psq_p.tile([128, 3 * HD], F32)  # 2 PSUM banks
                        for ct in range(CT):
                            ci, cj = ct // 4, ct % 4
                            lhs = xts[ci][:, cj, gi * 128:(gi + 1) * 128]
                            st, sp = ct == 0, ct == CT - 1
                            nc.tensor.matmul(
                                ps[:, 0:512], lhs, wq_t[ci][:, cj, 0:512],
                                start=st, stop=sp,
                            )
                            nc.tensor.matmul(
                                ps[:, 512:768], lhs,
                                wq_t[ci][:, cj, 512:768],
                                start=st, stop=sp,
                            )
                        # ---- epilogue: evacuate PSUM fast (ACT copy) ----
                        ev = evp.tile([128, 3 * HD], F32)
                        nc.vector.tensor_copy(ev, ps)
                        ssq = statp.tile([128, 8], F32)
                        sqscr = statp.tile([128, 128], F32, name="sqscr")
                        for b in range(6):   # q0 q1 k0 k1 v0 v1
                            nc.scalar.activation(
                                sqscr, ev[:, b * 128:(b + 1) * 128],
                                ACTF.Square, bias=zero_sb, scale=RSQRT_D,
                                accum_out=ssq[:, b:b + 1],
                            )
                        sq = statp.tile([128, 6], F32)
                        nc.scalar.activation(sq, ssq[:, 0:6], ACTF.Sqrt,
                                             bias=eps_sb)
                        rs = statp.tile([128, 6], F32)
                        nc.vector.reciprocal(rs, sq)
                        rs2 = statp.tile([128, 2], F32)
                        nc.vector.tensor_scalar_mul(rs2, rs[:, 4:6], lam_bc)

                        # ---- rope, batched across the 4 blocks q0 q1 k0 k1:
                        #   u = ev*[c|c] + ev_halfswapped*[s|-s]
                        c_bc = bass.AP(
                            tensor=cos_sb.tensor,
                            offset=cos_sb.offset + g * 128,
                            ap=[cos_sb.ap[0], [0, 4], [1, 128]],
                        )
                        s_bc = bass.AP(
                            tensor=sin_sb.tensor,
                            offset=sin_sb.offset + g * 128,
                            ap=[sin_sb.ap[0], [0, 4], [64, 2], [1, 64]],
                        )
                        ev_blk = ev[:, 0:512].rearrange(
                            "p (b j) -> p b j", b=4)
                        ev_swap = bass.AP(
                            tensor=ev.tensor, offset=ev.offset + 64,
                            ap=[ev.ap[0], [128, 4], [-64, 2], [1, 64]],
                        )
                        m1 = rp.tile([128, 512], F32, name="m1")
                        m2 = rp.tile([128, 512], F32, name="m2")
                        nc.vector.tensor_mul(
                            m1.rearrange("p (b j) -> p b j", b=4), ev_blk, c_bc)
                        nc.gpsimd.tensor_mul(
                            m2.rearrange("p (b s j) -> p b s j", b=4, s=2),
                            ev_swap, s_bc)
                        # u = m1 + m2 (in place into m1); qr = u * rs -> m2
                        nc.vector.tensor_add(m1, m1, m2)
                        qr = m2
                        for b in range(4):
                            nc.vector.tensor_scalar_mul(
                                qr[:, b * 128:(b + 1) * 128],
                                m1[:, b * 128:(b + 1) * 128],
                                rs[:, b:b + 1],
                            )
                        # ---- v mix ----
                        for h in range(HPC):
                            nc.vector.scalar_tensor_tensor(
                                out=v_t[g][:, h * 128:(h + 1) * 128],
                                in0=ev[:, 512 + h * 128:512 + (h + 1) * 128],
                                scalar=rs2[:, h:h + 1],
                                in1=ve_t[:, gi, h * 128:(h + 1) * 128],
                                op0=ALU.mult, op1=ALU.add,
                            )
                        # ---- transpose q,k to [d, t] ----
                        for b, (dst, h) in enumerate(
                                ((qT_t, 0), (qT_t, 1), (kT_t, 0), (kT_t, 1))):
                            tp = st_p.tile([128, 512], F32, name="tp", tag="stx")
                            nc.tensor.transpose(
                                tp[:, 0:128], qr[:, b * 128:(b + 1) * 128],
                                ident)
                            nc.vector.tensor_copy(
                                dst[h][g // 4][:, (g % 4) * 128:
                                               (g % 4 + 1) * 128],
                                tp[:, 0:128])

                def emit_p2p4(ch):
                    for h in range(HPC):
                        ntk = 4 * ch + 4
                        y_ps = y_p.tile([128, 512], F32)
                        l_ps = l_p.tile([1, 512], F32)
                        for tk in range(ntk):
                            off = max(0, tk * 128 - ch * 512)
                            stp = st_p.tile([128, 512], F32, tag="stx")
                            nc.tensor.matmul(
                                stp[:, off:512],
                                kT_t[h][tk // 4][:, (tk % 4) * 128:
                                                 (tk % 4 + 1) * 128],
                                qT_t[h][ch][:, off:512],
                                start=True, stop=True,
                            )
                            eT = e_p.tile([128, 512], MDT)
                            nc.scalar.activation(
                                eT[:, off:512], stp[:, off:512], ACTF.Exp,
                                bias=zero_sb, scale=ATTN_SCALE,
                            )
                            if tk * 128 >= ch * 512:  # diagonal block: mask
                                nc.vector.tensor_mul(
                                    eT[:, off:off + 128],
                                    eT[:, off:off + 128], mask_sb)
                            st, sp = tk == 0, tk == ntk - 1
                            nc.tensor.matmul(
                                y_ps[:, off:512],
                                v_t[tk][:, h * 128:(h + 1) * 128],
                                eT[:, off:512], start=st, stop=sp,
                            )
                            nc.tensor.matmul(
                                l_ps[:, off:512], ones_col, eT[:, off:512],
                                start=st, stop=sp,
                            )
                        rl = rl_p.tile([1, 512], MDT)
                        with nc.allow_low_precision(
                            reason="f32r reciprocal keeps 32-bit storage"
                        ):
                            nc.vector.reciprocal(rl, l_ps)
                        rb = st_p.tile([128, 512], F32, name="rb", tag="stx")
                        nc.tensor.matmul(rb, ones_row, rl,
                                         start=True, stop=True)
                        rb_sb = rl_p.tile([128, 512], F32, name="rb_sb")
                        nc.vector.tensor_copy(rb_sb, rb)
                        nc.vector.tensor_mul(
                            yT_t[h][ch], y_ps, rb_sb)
                        nc.sync.dma_start(
                            out=y_dram[ch][h * 128:(h + 1) * 128, :],
                            in_=yT_t[h][ch],
                        )

                    # ---- chunk AllGather ----
                    if sim_mode:
                        nc.sync.dma_start(
                            out=yfull[ch][0:HD, :], in_=y_dram[ch][:, :])
                    else:
                        nc.gpsimd.collective_compute(
                            "AllGather",
                            ALU.bypass,
                            replica_groups=rg,
                            ins=[y_dram[ch][:, :]],
                            outs=[yfull[ch][:, :]],
                        )

                    # ---- P4 chunk: outT[:, t-chunk] = WoT_loc.T @ y ----
                    o_ps0 = o_p.tile([128, 512], F32, name="ops0")
                    o_ps1 = o_p.tile([128, 512], F32, name="ops1")
                    o_ps = [o_ps0, o_ps1]
                    yfsrc = yfull[ch].rearrange("(a b p) t -> a p b t",
                                                a=8, b=2, p=128)
                    for hp in range(8):
                        yf = yf_p.tile([128, 2, 512], BF16)
                        nc.sync.dma_start(out=yf, in_=yfsrc[hp])
                        for hj in range(2):
                            ht = hp * 2 + hj
                            for c2 in range(2):
                                nc.tensor.matmul(
                                    o_ps[c2],
                                    wo_t[ht // 4][:, ht % 4,
                                                  c2 * 128:(c2 + 1) * 128],
                                    yf[:, hj, :],
                                    start=(ht == 0), stop=(ht == CT - 1),
                                )
                    ob = ob_p.tile([128, 2, 512], F32)
                    for c2 in range(2):
                        nc.vector.tensor_copy(ob[:, c2, :], o_ps[c2])
                    nc.sync.dma_start(
                        out=outT_d.rearrange("(c p) t -> p c t", p=128)[
                            :, :, ch * 512:(ch + 1) * 512],
                        in_=ob,
                    )

                # interleaved emission: P2 chunk ch right after the sg
                # that completes its inputs (kT/v up to tile 4ch+3)
                for sg in range(8):
                    emit_p1(sg)
                    if sg == 0:
                        for i in range(4):
                            nc.sync.dma_start(out=wo_t[i], in_=wo_src[i])
                    if sg % 2 == 1:
                        emit_p2p4(sg // 2)

    nc.finalize()
    return nc


_NC = None


def _get_nc():
    global _NC
    if _NC is None:
        _NC = _build_program()
    return _NC


def _rope_tables():
    quarter = D // 4
    af = (1.0 / 1024.0) ** np.linspace(0.0, 1.0, quarter, dtype=np.float32)
    af = np.concatenate([af, np.zeros(quarter, dtype=np.float32)])
    t = np.arange(T, dtype=np.float32)
    theta = t[:, None] * af[None, :]          # [T, 64]
    cos, sin = np.cos(theta), np.sin(theta)

    # [128, TT*128]; per t-tile block g: [cos | cos] and [sin | -sin]
    cos_t = cos.reshape(TT, 128, 64).astype(np.float32)
    sin_t = sin.reshape(TT, 128, 64).astype(np.float32)
    cos2 = np.concatenate([cos_t, cos_t], axis=2)       # [TT, 128, 128]
    sin2 = np.concatenate([sin_t, -sin_t], axis=2)

    def pack(m):
        return np.ascontiguousarray(
            m.transpose(1, 0, 2).reshape(128, TT * 128)).astype(np.float32)

    return pack(cos2), pack(sin2)


def _prepare_in_maps(x, qkvo_w, ve, lambdas):
    x = np.asarray(x, dtype=np.float32)
    qkvo_w = np.asarray(qkvo_w, dtype=np.float32)
    ve = np.asarray(ve, dtype=np.float32)
    lambdas = np.asarray(lambdas, dtype=np.float32)

    xT = np.ascontiguousarray(x[0].T)                      # [c, t]
    woT = np.ascontiguousarray(qkvo_w[3].T)                # [h, c_out]
    cos2_pk, sin2_pk = _rope_tables()

    p = np.arange(128)[:, None]
    j = np.arange(128)[None, :]
    masktri = (j >= p).astype(np.float32)

    lam_in = lambdas.reshape(1, 2)
    in_maps = []
    for i in range(NCORES):
        r = slice(HD * i, HD * (i + 1))
        wqkvT = np.ascontiguousarray(
            np.concatenate(
                [qkvo_w[0][r].T, qkvo_w[1][r].T, qkvo_w[2][r].T], axis=1
            )
        )                                                   # [c, 768]
        ve_s = np.ascontiguousarray(lambdas[1] * ve[0][:, r])  # [t, 256]
        woT_i = np.ascontiguousarray(woT[:, r]).astype(
            ml_dtypes.bfloat16)                                 # [h, 256]
        in_maps.append({
            "xT": xT,
            "wqkvT": wqkvT,
            "ve_s": ve_s,
            "woT": woT_i,
            "lam": lam_in,
            "cos2_pk": cos2_pk,
            "sin2_pk": sin2_pk,
            "masktri": masktri,
        })
    return in_maps


def _run_spmd(in_maps, trace=False):
    nc = _get_nc()
    return run_bass_kernel_spmd(
        nc, in_maps, core_ids=list(range(NCORES)), trace=trace
    )


def _assemble(results):
    out = np.empty((T, DIM), dtype=np.float32)
    for i in range(NCORES):
        out[:, HD * i:HD * (i + 1)] = results[i]["outT"].T
    return out.reshape(1, T, DIM)


def kernel(x, qkvo_w, ve, lambdas):
    in_maps = _prepare_in_maps(x, qkvo_w, ve, lambdas)
    res = _run_spmd(in_maps, trace=False)
    return _assemble(res.results)



# revision 11
# speedup vs baseline: 1.0723x; 1.0723x over previous
"""Trainium2 Bass/Tile kernel for causal self-attention (16 heads, T=2048, D=128).

Sharding: tensor-parallel over heads across 8 cores (2 heads/core).
 - QKV projection, RMSNorm, RoPE, lambda-mix, attention: per-head local.
 - y (pre-output-projection activations, [d, t] layout) AllGather'd across
   cores on the head axis, chunk-wise (4 x 512-column AllGathers) so the
   output projection pipelines behind attention.
 - Output projection sharded over output-channel columns: core i computes
   out[:, 256*i : 256*(i+1)] (transposed on device, un-transposed on host).

Matmuls run in float32r (fp32 storage, PE rounds to ~11-bit mantissa,
1 cyc/row at N>=256) — measured rel err ~1.5e-4 per dot on HW.

Softmax uses no max-subtraction: RMS-normed q,k bound |scores| <= 0.12*128,
so exp never overflows fp32; row sums are produced by a ones-vector matmul
and divided out at the end (flash-attention style).

Self-contained: hardcodes all shapes; no reads of /root/problem/*.
"""

from contextlib import ExitStack

import ml_dtypes
import numpy as np

import concourse.bass as bass
import concourse.mybir as mybir
import concourse.tile as tile
from concourse import bacc
from concourse.bass_utils import run_bass_kernel_spmd
from concourse.masks import make_identity

F32 = mybir.dt.float32
F32R = mybir.dt.float32r
BF16 = mybir.dt.bfloat16
MDT = BF16  # matmul operand dtype (q/k/v/score path)
ALU = mybir.AluOpType
ACTF = mybir.ActivationFunctionType

T = 2048          # sequence length
DIM = 2048        # model dim
D = 128           # head dim
NCORES = 8
HPC = 2           # heads per core
HD = HPC * D      # local head-dim cols (256)
TT = T // 128     # t tiles (16)
CT = DIM // 128   # c tiles (16)
NCH = T // 512    # 512-wide t chunks (4)
ATTN_SCALE = 0.12
EPS = 1e-6
RSQRT_D = float(1.0 / np.sqrt(128.0))


def _build_program(sim_mode=False):
    nc = bacc.Bacc(
        "TRN2",
        target_bir_lowering=False,
        debug=False,
        num_devices=NCORES,
    )

    xT_d = nc.dram_tensor("xT", [DIM, T], MDT, kind="ExternalInput")
    wqkvT_d = nc.dram_tensor("wqkvT", [DIM, 3 * HD], MDT, kind="ExternalInput")
    ve_s_d = nc.dram_tensor("ve_s", [T, HD], BF16, kind="ExternalInput")
    woT_d = nc.dram_tensor("woT", [DIM, HD], BF16, kind="ExternalInput")
    lam_d = nc.dram_tensor("lam", [1, 2], F32, kind="ExternalInput")
    cos_d = nc.dram_tensor("cos2_pk", [128, TT * 128], BF16,
                           kind="ExternalInput")
    sin_d = nc.dram_tensor("sin2_pk", [128, TT * 128], BF16,
                           kind="ExternalInput")
    mask_d = nc.dram_tensor("masktri", [128, 128], MDT, kind="ExternalInput")
    outT_d = nc.dram_tensor("outT", [HD, T], F32, kind="ExternalOutput")

    rg = [list(range(NCORES))]

    with tile.TileContext(nc) as tc:
        with (
            tc.tile_pool(name="const", bufs=1) as constp,
            tc.tile_pool(name="persist", bufs=1) as persist,
            tc.tile_pool(name="wqkv", bufs=1) as wqp,
            tc.tile_pool(name="wo", bufs=1) as wop,
            tc.tile_pool(name="dram", bufs=1, space="DRAM") as dramp,
        ):
            # ---- constants (cos/sin DMAs deferred past first weights) ----
            cos_sb = constp.tile([128, TT * 128], BF16)
            sin_sb = constp.tile([128, TT * 128], BF16)
            mask_sb = constp.tile([128, 128], MDT)
            ones_f32 = constp.tile([128, 128], F32)
            nc.vector.memset(ones_f32, 1.0)
            ones_col = constp.tile([128, 1], MDT)
            nc.vector.tensor_copy(ones_col, ones_f32[:, 0:1])
            ones_row = constp.tile([1, 128], F32R)
            nc.vector.tensor_copy(ones_row, ones_f32[0:1, :])
            ident = constp.tile([128, 128], BF16)
            make_identity(nc, ident)
            lam_bc = constp.tile([128, 1], F32)
            nc.sync.dma_start(out=lam_bc, in_=lam_d[0:1, 0:1].to_broadcast([128, 1]))
            eps_sb = constp.tile([128, 1], F32)
            nc.vector.memset(eps_sb, EPS)
            zero_sb = constp.tile([128, 1], F32)
            nc.vector.memset(zero_sb, 0.0)

            # ---- weight tiles (coalesced: 4 cts per DMA) ----
            wq_t = [wqp.tile([128, 4, 3 * HD], MDT, name=f"wq{i}")
                    for i in range(4)]
            wq_src = wqkvT_d.rearrange("(a b p) e -> a p b e", a=4, b=4, p=128)

            # ---- persistent activations (per-region tiles) ----
            qT_t = [[persist.tile([128, 512], MDT, name=f"qT{h}_{c}")
                     for c in range(NCH)] for h in range(HPC)]
            kT_t = [[persist.tile([128, 512], MDT, name=f"kT{h}_{c}")
                     for c in range(NCH)] for h in range(HPC)]
            v_t = [persist.tile([128, HD], MDT, name=f"v{g}")
                   for g in range(TT)]
            yT_t = [[persist.tile([128, 512], BF16, name=f"yT{h}_{c}")
                     for c in range(NCH)] for h in range(HPC)]

            # per-chunk DRAM tiles for the pipelined AllGather
            y_dram = [dramp.tile([HD, 512], BF16, name=f"ydr{c}")
                      for c in range(NCH)]
            yfull = [dramp.tile([DIM, 512], BF16, addr_space="Shared",
                                name=f"yfl{c}")
                     for c in range(NCH)]

            # =========== P1: qkv projection + norm + rope + mix =============
            with ExitStack() as pools:
                ep = pools.enter_context
                xtp = ep(tc.tile_pool(name="p1x", bufs=1))
                vep = ep(tc.tile_pool(name="p1ve", bufs=2))
                psq_p = ep(tc.tile_pool(name="p1ps", bufs=1, space="PSUM"))
                evp = ep(tc.tile_pool(name="p1ev", bufs=3))
                statp = ep(tc.tile_pool(name="p1stat", bufs=4))
                rp = ep(tc.tile_pool(name="p1r", bufs=2))
                st_p = ep(tc.tile_pool(name="p2st", bufs=2, space="PSUM"))
                y_p = ep(tc.tile_pool(name="p2y", bufs=1, space="PSUM"))
                l_p = ep(tc.tile_pool(name="p2l", bufs=1, space="PSUM"))
                o_p = ep(tc.tile_pool(name="p4ps", bufs=1, space="PSUM"))
                e_p = ep(tc.tile_pool(name="p2e", bufs=4))
                rl_p = ep(tc.tile_pool(name="p2rl", bufs=2))
                yf_p = ep(tc.tile_pool(name="p4yf", bufs=3))
                ob_p = ep(tc.tile_pool(name="p4ob", bufs=2))
                # output-projection weights (first needed by P4 chunk 0)
                wo_t = [wop.tile([128, 4, HD], BF16, name=f"wo{i}")
                        for i in range(4)]
                wo_src = woT_d.rearrange("(a b p) e -> a p b e", a=4, b=4,
                                         p=128)

                def emit_p1(sg):
                    xts = []
                    xt_src = xT_d.rearrange(
                        "(ci cj p) t -> ci p cj t", ci=4, cj=4, p=128)
                    for ci in range(4):
                        xt = xtp.tile([128, 4, 256], MDT, name=f"xt{ci}")
                        if sg == 0 and ci == 0:
                            # fine-grained first group so the PE starts ASAP
                            for cj in range(4):
                                nc.sync.dma_start(
                                    out=wq_t[0][:, cj, :],
                                    in_=wq_src[0][:, cj, :])
                                nc.sync.dma_start(
                                    out=xt[:, cj, :],
                                    in_=xt_src[0, :, cj, 0:256])
                        else:
                            if sg == 0:
                                nc.sync.dma_start(out=wq_t[ci],
                                                  in_=wq_src[ci])
                            nc.sync.dma_start(
                                out=xt,
                                in_=xt_src[ci, :, :, sg * 256:(sg + 1) * 256],
                            )
                        xts.append(xt)
                    if sg == 0:
                        nc.sync.dma_start(out=cos_sb, in_=cos_d[:, :])
                        nc.sync.dma_start(out=sin_sb, in_=sin_d[:, :])
                        nc.sync.dma_start(out=mask_sb, in_=mask_d[:, :])
                    ve_t = vep.tile([128, 2, HD], BF16, name="ve_t")
                    nc.sync.dma_start(
                        out=ve_t,
                        in_=ve_s_d.rearrange("(a two p) d -> a p two d",
                                             two=2, p=128)[sg],
                    )
                    for gi in range(2):
                        g = sg * 2 + gi
                        ps = psq_p.tile([128, 3 * HD], F32)  # 2 PSUM banks
                        for ct in range(CT):
                            ci, cj = ct // 4, ct % 4
                            lhs = xts[ci][:, cj, gi * 128:(gi + 1) * 128]
                            st, sp = ct == 0, ct == CT - 1
                            nc.tensor.matmul(
                                ps[:, 0:512], lhs, wq_t[ci][:, cj, 0:512],
                                start=st, stop=sp,
                            )
                            nc.tensor.matmul(
                                ps[:, 512:768], lhs,
                                wq_t[ci][:, cj, 512:768],
                                start=st, stop=sp,
                            )
                        # ---- epilogue: evacuate PSUM fast (ACT copy) ----
                        ev = evp.tile([128, 3 * HD], BF16)
                        nc.vector.tensor_copy(ev, ps)
                        ssq = statp.tile([128, 8], F32)
                        sqscr = statp.tile([128, 128], BF16, name="sqscr")
                        for b in range(6):   # q0 q1 k0 k1 v0 v1
                            nc.scalar.activation(
                                sqscr, ev[:, b * 128:(b + 1) * 128],
                                ACTF.Square, bias=zero_sb, scale=RSQRT_D,
                                accum_out=ssq[:, b:b + 1],
                            )
                        sq = statp.tile([128, 6], F32)
                        nc.scalar.activation(sq, ssq[:, 0:6], ACTF.Sqrt,
                                             bias=eps_sb)
                        rs = statp.tile([128, 6], F32)
                        nc.vector.reciprocal(rs, sq)
                        rs2 = statp.tile([128, 2], F32)
                        nc.vector.tensor_scalar_mul(rs2, rs[:, 4:6], lam_bc)

                        # ---- rope, batched across the 4 blocks q0 q1 k0 k1:
                        #   u = ev*[c|c] + ev_halfswapped*[s|-s]
                        c_bc = bass.AP(
                            tensor=cos_sb.tensor,
                            offset=cos_sb.offset + g * 128,
                            ap=[cos_sb.ap[0], [0, 4], [1, 128]],
                        )
                        s_bc = bass.AP(
                            tensor=sin_sb.tensor,
                            offset=sin_sb.offset + g * 128,
                            ap=[sin_sb.ap[0], [0, 4], [64, 2], [1, 64]],
                        )
                        ev_blk = ev[:, 0:512].rearrange(
                            "p (b j) -> p b j", b=4)
                        ev_swap = bass.AP(
                            tensor=ev.tensor, offset=ev.offset + 64,
                            ap=[ev.ap[0], [128, 4], [-64, 2], [1, 64]],
                        )
                        m1 = rp.tile([128, 512], BF16, name="m1")
                        m2 = rp.tile([128, 512], BF16, name="m2")
                        nc.vector.tensor_mul(
                            m1.rearrange("p (b j) -> p b j", b=4), ev_blk, c_bc)
                        nc.gpsimd.tensor_mul(
                            m2.rearrange("p (b s j) -> p b s j", b=4, s=2),
                            ev_swap, s_bc)
                        # u = m1 + m2 (in place into m1); qr = u * rs -> m2
                        nc.vector.tensor_add(m1, m1, m2)
                        qr = m2
                        for b in range(4):
                            nc.vector.tensor_scalar_mul(
                                qr[:, b * 128:(b + 1) * 128],
                                m1[:, b * 128:(b + 1) * 128],
                                rs[:, b:b + 1],
                            )
                        # ---- v mix ----
                        for h in range(HPC):
                            nc.vector.scalar_tensor_tensor(
                                out=v_t[g][:, h * 128:(h + 1) * 128],
                                in0=ev[:, 512 + h * 128:512 + (h + 1) * 128],
                                scalar=rs2[:, h:h + 1],
                                in1=ve_t[:, gi, h * 128:(h + 1) * 128],
                                op0=ALU.mult, op1=ALU.add,
                            )
                        # ---- transpose q,k to [d, t] ----
                        for b, (dst, h) in enumerate(
                                ((qT_t, 0), (qT_t, 1), (kT_t, 0), (kT_t, 1))):
                            tp = st_p.tile([128, 512], BF16, name="tp",
                                           tag="stx")
                            nc.tensor.transpose(
                                tp[:, 0:128], qr[:, b * 128:(b + 1) * 128],
                                ident)
                            nc.vector.tensor_copy(
                                dst[h][g // 4][:, (g % 4) * 128:
                                               (g % 4 + 1) * 128],
                                tp[:, 0:128])

                def emit_p2p4(ch):
                    for h in range(HPC):
                        ntk = 4 * ch + 4
                        y_ps = y_p.tile([128, 512], F32)
                        l_ps = l_p.tile([1, 512], F32)
                        for tk in range(ntk):
                            off = max(0, tk * 128 - ch * 512)
                            stp = st_p.tile([128, 512], F32, tag="stx")
                            nc.tensor.matmul(
                                stp[:, off:512],
                                kT_t[h][tk // 4][:, (tk % 4) * 128:
                                                 (tk % 4 + 1) * 128],
                                qT_t[h][ch][:, off:512],
                                start=True, stop=True,
                            )
                            eT = e_p.tile([128, 512], MDT)
                            nc.scalar.activation(
                                eT[:, off:512], stp[:, off:512], ACTF.Exp,
                                bias=zero_sb, scale=ATTN_SCALE,
                            )
                            if tk * 128 >= ch * 512:  # diagonal block: mask
                                nc.vector.tensor_mul(
                                    eT[:, off:off + 128],
                                    eT[:, off:off + 128], mask_sb)
                            st, sp = tk == 0, tk == ntk - 1
                            nc.tensor.matmul(
                                y_ps[:, off:512],
                                v_t[tk][:, h * 128:(h + 1) * 128],
                                eT[:, off:512], start=st, stop=sp,
                            )
                            nc.tensor.matmul(
                                l_ps[:, off:512], ones_col, eT[:, off:512],
                                start=st, stop=sp,
                            )
                        rl = rl_p.tile([1, 512], F32R)
                        with nc.allow_low_precision(
                            reason="f32r reciprocal keeps 32-bit storage"
                        ):
                            nc.vector.reciprocal(rl, l_ps)
                        rb = st_p.tile([128, 512], F32, name="rb", tag="stx")
                        nc.tensor.matmul(rb, ones_row, rl,
                                         start=True, stop=True)
                        rb_sb = rl_p.tile([128, 512], F32, name="rb_sb")
                        nc.vector.tensor_copy(rb_sb, rb)
                        nc.vector.tensor_mul(
                            yT_t[h][ch], y_ps, rb_sb)
                        nc.sync.dma_start(
                            out=y_dram[ch][h * 128:(h + 1) * 128, :],
                            in_=yT_t[h][ch],
                        )

                    # ---- chunk AllGather ----
                    if sim_mode:
                        nc.sync.dma_start(
                            out=yfull[ch][0:HD, :], in_=y_dram[ch][:, :])
                    else:
                        nc.gpsimd.collective_compute(
                            "AllGather",
                            ALU.bypass,
                            replica_groups=rg,
                            ins=[y_dram[ch][:, :]],
                            outs=[yfull[ch][:, :]],
                        )

                    # ---- P4 chunk: outT[:, t-chunk] = WoT_loc.T @ y ----
                    o_ps0 = o_p.tile([128, 512], F32, name="ops0")
                    o_ps1 = o_p.tile([128, 512], F32, name="ops1")
                    o_ps = [o_ps0, o_ps1]
                    yfsrc = yfull[ch].rearrange("(a b p) t -> a p b t",
                                                a=8, b=2, p=128)
                    for hp in range(8):
                        yf = yf_p.tile([128, 2, 512], BF16)
                        nc.sync.dma_start(out=yf, in_=yfsrc[hp])
                        for hj in range(2):
                            ht = hp * 2 + hj
                            for c2 in range(2):
                                nc.tensor.matmul(
                                    o_ps[c2],
                                    wo_t[ht // 4][:, ht % 4,
                                                  c2 * 128:(c2 + 1) * 128],
                                    yf[:, hj, :],
                                    start=(ht == 0), stop=(ht == CT - 1),
                                )
                    ob = ob_p.tile([128, 2, 512], F32)
                    for c2 in range(2):
                        nc.vector.tensor_copy(ob[:, c2, :], o_ps[c2])
                    nc.sync.dma_start(
                        out=outT_d.rearrange("(c p) t -> p c t", p=128)[
                            :, :, ch * 512:(ch + 1) * 512],
                        in_=ob,
                    )

                # interleaved emission: P2 chunk ch right after the sg
                # that completes its inputs (kT/v up to tile 4ch+3)
                for sg in range(8):
                    emit_p1(sg)
                    if sg == 0:
                        for i in range(4):
                            nc.sync.dma_start(out=wo_t[i], in_=wo_src[i])
                    if sg % 2 == 1:
                        emit_p2p4(sg // 2)

    nc.finalize()
    return nc


_NC = None


def _get_nc():
    global _NC
    if _NC is None:
        _NC = _build_program()
    return _NC


def _rope_tables():
    quarter = D // 4
    af = (1.0 / 1024.0) ** np.linspace(0.0, 1.0, quarter, dtype=np.float32)
    af = np.concatenate([af, np.zeros(quarter, dtype=np.float32)])
    t = np.arange(T, dtype=np.float32)
    theta = t[:, None] * af[None, :]          # [T, 64]
    cos, sin = np.cos(theta), np.sin(theta)

    # [128, TT*128]; per t-tile block g: [cos | cos] and [sin | -sin]
    cos_t = cos.reshape(TT, 128, 64).astype(np.float32)
    sin_t = sin.reshape(TT, 128, 64).astype(np.float32)
    cos2 = np.concatenate([cos_t, cos_t], axis=2)       # [TT, 128, 128]
    sin2 = np.concatenate([sin_t, -sin_t], axis=2)

    def pack(m):
        return np.ascontiguousarray(
            m.transpose(1, 0, 2).reshape(128, TT * 128)).astype(
                ml_dtypes.bfloat16)

    return pack(cos2), pack(sin2)


def _prepare_in_maps(x, qkvo_w, ve, lambdas):
    x = np.asarray(x, dtype=np.float32)
    qkvo_w = np.asarray(qkvo_w, dtype=np.float32)
    ve = np.asarray(ve, dtype=np.float32)
    lambdas = np.asarray(lambdas, dtype=np.float32)

    xT = np.ascontiguousarray(x[0].T).astype(ml_dtypes.bfloat16)  # [c, t]
    woT = np.ascontiguousarray(qkvo_w[3].T)                # [h, c_out]
    cos2_pk, sin2_pk = _rope_tables()

    p = np.arange(128)[:, None]
    j = np.arange(128)[None, :]
    masktri = (j >= p).astype(ml_dtypes.bfloat16)

    lam_in = lambdas.reshape(1, 2)
    in_maps = []
    for i in range(NCORES):
        r = slice(HD * i, HD * (i + 1))
        wqkvT = np.ascontiguousarray(
            np.concatenate(
                [qkvo_w[0][r].T, qkvo_w[1][r].T, qkvo_w[2][r].T], axis=1
            )
        ).astype(ml_dtypes.bfloat16)                        # [c, 768]
        ve_s = np.ascontiguousarray(lambdas[1] * ve[0][:, r]).astype(
            ml_dtypes.bfloat16)                             # [t, 256]
        woT_i = np.ascontiguousarray(woT[:, r]).astype(
            ml_dtypes.bfloat16)                                 # [h, 256]
        in_maps.append({
            "xT": xT,
            "wqkvT": wqkvT,
            "ve_s": ve_s,
            "woT": woT_i,
            "lam": lam_in,
            "cos2_pk": cos2_pk,
            "sin2_pk": sin2_pk,
            "masktri": masktri,
        })
    return in_maps


def _run_spmd(in_maps, trace=False):
    nc = _get_nc()
    return run_bass_kernel_spmd(
        nc, in_maps, core_ids=list(range(NCORES)), trace=trace
    )


def _assemble(results):
    out = np.empty((T, DIM), dtype=np.float32)
    for i in range(NCORES):
        out[:, HD * i:HD * (i + 1)] = results[i]["outT"].T
    return out.reshape(1, T, DIM)


def kernel(x, qkvo_w, ve, lambdas):
    in_maps = _prepare_in_maps(x, qkvo_w, ve, lambdas)
    res = _run_spmd(in_maps, trace=False)
    return _assemble(res.results)

